# revision 55
# baseline (speedup 1.0000x reference)
"""Trainium2 Bass kernel for a custom-activation LSTM cell.

  gates = (x @ w_ih.T + b_ih) + (h @ w_hh.T + b_hh)   # [B, 4H], gate order f,i,ic,o
  ft, it, ot = sigmoid(...), i_cands = sin(ic_in)
  ct = c*ft + sin(ic_in)*it ; ht = sigmoid(o_in)*sin(ct)

Sharding: each of the 8 cores computes the SAME 256-wide slice of H for all
four gates (rows g*2048 + core*256 .. +256 of the weight matrices). Each core
then owns columns [core*256, (core+1)*256) of ht/ct for the full batch — no
cross-core communication is needed.

Layout: compute is done transposed — out[g_tile, batch] = wT.T @ actT — so the
host pre-transposes x/h/c and the per-core weight slices once (cheap, shared
across cores for x/h/c), and the device kernel does only contiguous DMA.

Precision (per-gate, see _build_module_pg): the sin-candidate gate (ic) is
2-4x more error-sensitive than the sigmoid gates, so the fp8 budget goes to
the others: f and i run ALL 24 k-tiles as fp8-e4m3 perf_mode=DoubleRow
matmuls (two k-tiles folded per MM, ~1.8x the bf16 MAC rate), o runs its
first 4 x k-tiles in fp8 DR, ic stays fully bf16. All gate weights are
pre-scaled by WS=256 on the host so fp8 weight values (std ~5) sit far from
the e4m3 subnormal floor; the 1/WS descale folds into the ACT scale for the
sigmoid gates and into one DVE tensor_scalar for the sin gate. Measured
output rel_fro: concat 1.604e-2, ht 1.664e-2, ct 1.593e-2 (gate 2e-2);
all-fp8 measures 3.1e-2 and fails. Device fp8 results match an ml_dtypes
host simulation bit-for-bit at these magnitudes.

DMA diet: ablations (no_act_dma) showed the MM streams at full pitch but
the full kernel DMA-stalled at 6MB/chunk. Activations now stream bf16-only;
the fp8 copies are cast on-device (xc8 on ACT, hc8 on DVE), software-
pipelined one chunk ahead via alloc_chunk(); c/ht/ct ride bf16. 3.75MB/chunk
total, worth ~50us/iter.

sigmoid is computed as 0.5*tanh(0.5x + 0.5b) + 0.5 so that tanh and sin come
from ONE ACT table set (silu_and_others) — a raw sigmoid would force a ~2.7us
table switch per tile against sin.

MM-stream ordering (hardware-measured on this device):
  - bf16 k-outer over 8 banks: ~216 ns/MM; gate-serial same-bank runs:
    ~236 ns/MM. Both fine.
  - m-outer (8 consecutive same-bank MMs w/ start-of-group) and 2-/4-bank
    bf16 alternation after DR blocks: +130-170 ns/MM. Avoid.
  - fp8 DR blocks at 4-8 bank rotation: ~241 ns/MM covering 2 k-tiles. Good.
So: DR blocks run k-outer m-inner; the bf16 ic/o blocks run gate-serial
(serial_bf16=True default).
"""

import ml_dtypes
import numpy as np

import concourse.bass as bass
import concourse.tile as tile
from concourse import bacc, mybir
from concourse.bass_utils import run_bass_kernel_spmd

# Problem shapes (hardcoded per the harness contract).
B, IN, H = 4096, 1024, 2048
NCORES = 8
P = 128
SH = H // NCORES          # 256  H-slice per core
G = 4 * SH                # 1024 gate rows per core (f,i,ic,o x 256)
MT = G // P               # 8 m-tiles: [f0 i0 ic0 o0 f1 i1 ic1 o1]
KX = IN // P              # 8 k-tiles from x
NFP8 = 6                  # x k-tiles in fp8 DoubleRow (must be even)
NXB = KX - NFP8           # 2 x k-tiles kept bf16
KH = H // P               # 16 k-tiles from h
NB = 512                  # batch chunk (matmul moving dim)
NBCH = B // NB            # 8 chunks
WS = 256.0                # weight scale (fp8 range), descaled in elementwise

F32 = mybir.dt.float32
BF16 = mybir.dt.bfloat16
F8E4 = mybir.dt.float8e4
ACT = mybir.ActivationFunctionType
DR = mybir.MatmulPerfMode.DoubleRow

_MODULES: dict[int, "bacc.Bacc"] = {}


def _build_module(repeats: int = 1, lead_silu: bool = True,
                  internal_io: bool = False, unroll: bool = False,
                  nfp8: int = NFP8, grouped: bool = False,
                  xorder: str = "m_outer") -> "bacc.Bacc":
    """Build + compile the per-core Bass module.

    repeats > 1 wraps the whole compute in a hardware loop (used only for
    timing: the per-iteration device time is (T(R) - T(1)) / (R - 1)).
    nfp8/grouped/xorder are timing-experiment knobs (see x-phase below)."""
    nc = bacc.Bacc("TRN2", target_bir_lowering=False, debug=False,
                   num_devices=NCORES)

    # internal_io=True is a timing-only variant: the big tensors live in
    # Internal DRAM (uninitialized, never uploaded/downloaded) so the
    # per-call wall time is not dominated by host<->device transfers.
    kin = "Internal" if internal_io else "ExternalInput"
    kout = "Internal" if internal_io else "ExternalOutput"

    nxb = KX - nfp8
    x8T3 = xbT3 = wih83 = wihb3 = None
    if nfp8:
        x8T = nc.dram_tensor("x8T", [nfp8 * P, B], F8E4, kind=kin).ap()
        wih8 = nc.dram_tensor("wih8", [nfp8 * P, G], F8E4, kind=kin).ap()
        x8T3 = x8T.rearrange("(ko p) b -> p ko b", p=P)    # [128, nfp8, B]
        wih83 = wih8.rearrange("(ko p) g -> p ko g", p=P)  # [128, nfp8, G]
    if nxb:
        xbT = nc.dram_tensor("xbT", [nxb * P, B], BF16, kind=kin).ap()
        wihb = nc.dram_tensor("wihb", [nxb * P, G], BF16, kind=kin).ap()
        xbT3 = xbT.rearrange("(ko p) b -> p ko b", p=P)    # [128, nxb, B]
        wihb3 = wihb.rearrange("(ko p) g -> p ko g", p=P)  # [128, nxb, G]
    hT = nc.dram_tensor("hT", [H, B], BF16, kind=kin).ap()
    cT = nc.dram_tensor("cT", [SH, B], F32, kind=kin).ap()
    whh = nc.dram_tensor("whh", [H, G], BF16, kind=kin).ap()
    biasd = nc.dram_tensor("biasd", [P, MT], F32, kind="ExternalInput").ap()
    htT = nc.dram_tensor("htT", [SH, B], F32, kind=kout).ap()
    ctT = nc.dram_tensor("ctT", [SH, B], F32, kind=kout).ap()

    hT3 = hT.rearrange("(ko p) b -> p ko b", p=P)      # [128, KH, B]
    cT3 = cT.rearrange("(po p) b -> p po b", p=P)      # [128, 2, B]
    whh3 = whh.rearrange("(ko p) g -> p ko g", p=P)    # [128, KH, G]
    htT3 = htT.rearrange("(po p) b -> p po b", p=P)
    ctT3 = ctT.rearrange("(po p) b -> p po b", p=P)

    with tile.TileContext(nc) as tc:
        with (
            tc.tile_pool(name="wpool", bufs=1) as wpool,
            tc.tile_pool(name="apool", bufs=2) as apool,
            tc.tile_pool(name="gpool", bufs=2) as gpool,
            tc.tile_pool(name="opool", bufs=3) as opool,
            tc.tile_pool(name="pspool", bufs=1, space="PSUM") as pspool,
        ):
            # Weights + bias resident in SBUF for the whole kernel, loaded
            # ONCE outside the (timing) repeat loop — the graded metric is the
            # steady-state per-iteration time with weights resident. Host lays
            # out G as [ph=0 gates f,i,ic,o | ph=1 gates f,i,ic,o]
            # (m = gi + 4*ph).
            w_ih8_t = w_ihb_t = None
            if nfp8:
                w_ih8_t = wpool.tile([P, nfp8, G], F8E4, tag="wih8")
                for k in range(nfp8):
                    nc.sync.dma_start(out=w_ih8_t[:, k, :], in_=wih83[:, k, :])
            if nxb:
                w_ihb_t = wpool.tile([P, nxb, G], BF16, tag="wihb")
                for k in range(nxb):
                    nc.sync.dma_start(out=w_ihb_t[:, k, :], in_=wihb3[:, k, :])
            w_hh_t = wpool.tile([P, KH, G], BF16, tag="whh")
            bias_sb = wpool.tile([P, MT], F32)
            for k in range(KH):
                nc.sync.dma_start(out=w_hh_t[:, k, :], in_=whh3[:, k, :])
            nc.sync.dma_start(out=bias_sb, in_=biasd)

            # Dummy Silu: forces the ACT table loader to pick the
            # silu_and_others set (the only one containing BOTH Tanh and
            # Sin), so the whole kernel needs exactly one table load.
            # Without it the loader ping-pongs exp_and_others (Tanh) and
            # trig_and_small (Sin) at ~2.7us per switch, 64 times.
            if lead_silu:
                dummy = wpool.tile([P, 1], F32)
                nc.vector.memset(dummy, 0.0)
                nc.scalar.activation(dummy, dummy, ACT.Silu)

            def elementwise(ph, ps, cc, bsl):
                cols = [gi + 4 * ph for gi in range(4)]
                PI, TWO_PI = float(np.pi), float(2 * np.pi)
                ft = gpool.tile([P, NB], F32, tag="ft")
                it = gpool.tile([P, NB], F32, tag="it")
                gt = gpool.tile([P, NB], F32, tag="gt")
                ot = gpool.tile([P, NB], F32, tag="ot")
                # PSUM holds WS*(z); sigmoid(z+b) = 0.5*tanh(0.5z + 0.5b)+0.5
                # (bias column for tanh gates is pre-scaled by 0.5; the 1/WS
                # descale folds into the ACT input scale).
                nc.scalar.activation(ft, ps[4 * ph + 0], ACT.Tanh,
                                     bias=bias_sb[:, cols[0]:cols[0] + 1],
                                     scale=0.5 / WS)
                nc.scalar.activation(it, ps[4 * ph + 1], ACT.Tanh,
                                     bias=bias_sb[:, cols[1]:cols[1] + 1],
                                     scale=0.5 / WS)
                # ACT Sin is only valid on [-pi, pi]; descale+bias on the DVE
                # first, then wrap by one 2*pi period.
                MUL, ADD = mybir.AluOpType.mult, mybir.AluOpType.add
                gw1 = gpool.tile([P, NB], F32, tag="gw1")
                nc.vector.tensor_scalar(
                    gw1, ps[4 * ph + 2], 1.0 / WS,
                    bias_sb[:, cols[2]:cols[2] + 1], MUL, ADD)
                gw = gpool.tile([P, NB], F32, tag="gw")
                nc.vector.add_range_wrap(gw, gw1, 0.0, PI, TWO_PI)
                nc.scalar.activation(gt, gw, ACT.Sin)
                nc.scalar.activation(ot, ps[4 * ph + 3], ACT.Tanh,
                                     bias=bias_sb[:, cols[3]:cols[3] + 1],
                                     scale=0.5 / WS)
                nc.vector.tensor_scalar(ft, ft, 0.5, 0.5, MUL, ADD)
                nc.vector.tensor_scalar(it, it, 0.5, 0.5, MUL, ADD)
                nc.vector.tensor_scalar(ot, ot, 0.5, 0.5, MUL, ADD)

                ctn = opool.tile([P, NB], F32, tag="ctn")
                tmp = opool.tile([P, NB], F32, tag="tmp")
                nc.vector.tensor_mul(ctn, cc[:, ph, :], ft)
                nc.vector.tensor_mul(tmp, gt, it)
                nc.vector.tensor_add(ctn, ctn, tmp)
                cw = opool.tile([P, NB], F32, tag="cw")
                nc.vector.add_range_wrap(cw, ctn, 0.0, PI, TWO_PI)
                sct = opool.tile([P, NB], F32, tag="sct")
                nc.scalar.activation(sct, cw, ACT.Sin)
                htn = opool.tile([P, NB], F32, tag="htn")
                nc.vector.tensor_mul(htn, ot, sct)
                nc.sync.dma_start(out=ctT3[:, ph, bsl], in_=ctn)
                nc.sync.dma_start(out=htT3[:, ph, bsl], in_=htn)

            def dr_mm(ps_m, m, kp, xc8, start):
                msl = bass.ds(m * P, P)
                nc.tensor.matmul(
                    ps_m, lhsT=w_ih8_t[:, 2 * kp:2 * kp + 2, msl],
                    rhs=xc8[:, 2 * kp:2 * kp + 2, :],
                    start=start, stop=False, perf_mode=DR)

            def xb_mm(ps_m, m, kb, xcb, start):
                msl = bass.ds(m * P, P)
                nc.tensor.matmul(
                    ps_m, lhsT=w_ihb_t[:, kb, msl], rhs=xcb[:, kb, :],
                    start=start, stop=False)

            def body():
                for nb in range(NBCH):
                    bsl = bass.ds(nb * NB, NB)
                    hc = apool.tile([P, KH, NB], BF16, tag="hc")
                    cc = apool.tile([P, 2, NB], F32, tag="cc")
                    xc8 = xcb = None
                    if nfp8:
                        xc8 = apool.tile([P, nfp8, NB], F8E4, tag="xc8")
                        nc.sync.dma_start(out=xc8, in_=x8T3[:, :, bsl])
                    if nxb:
                        xcb = apool.tile([P, nxb, NB], BF16, tag="xcb")
                        nc.sync.dma_start(out=xcb, in_=xbT3[:, :, bsl])
                    nc.sync.dma_start(out=hc, in_=hT3[:, :, bsl])
                    nc.sync.dma_start(out=cc, in_=cT3[:, :, bsl])

                    ps = [pspool.tile([P, NB], F32, tag=f"ps{m}", name=f"ps{m}")
                          for m in range(MT)]
                    # x-phase, m-outer: each PSUM bank gets all its x-work
                    # before the next bank is claimed (see module docstring).
                    # grouped=True runs all DoubleRow MMs for all m-tiles,
                    # then all bf16 x MMs (2 perf-mode switches per chunk
                    # instead of 2 per m-tile).
                    if xorder == "k_outer":
                        for kp in range(nfp8 // 2):
                            for m in range(MT):
                                dr_mm(ps[m], m, kp, xc8, kp == 0)
                        for kb in range(nxb):
                            for m in range(MT):
                                xb_mm(ps[m], m, kb, xcb, nfp8 == 0 and kb == 0)
                    elif grouped:
                        for m in range(MT):
                            for kp in range(nfp8 // 2):
                                dr_mm(ps[m], m, kp, xc8, kp == 0)
                        for m in range(MT):
                            for kb in range(nxb):
                                xb_mm(ps[m], m, kb, xcb,
                                      nfp8 == 0 and kb == 0)
                    else:
                        for m in range(MT):
                            for kp in range(nfp8 // 2):
                                dr_mm(ps[m], m, kp, xc8, kp == 0)
                            for kb in range(nxb):
                                xb_mm(ps[m], m, kb, xcb,
                                      nfp8 == 0 and kb == 0)
                    # h-phase per ph-half, gate-serial: each gate's
                    # accumulation stops early so its ACT work drains while
                    # later gates still matmul (shrinks the elementwise tail).
                    for ph in range(2):
                        for gi in range(4):
                            m = 4 * ph + gi
                            for k in range(KH):
                                nc.tensor.matmul(
                                    ps[m],
                                    lhsT=w_hh_t[:, k, bass.ds(m * P, P)],
                                    rhs=hc[:, k, :],
                                    start=False, stop=(k == KH - 1),
                                )
                        elementwise(ph, ps, cc, bsl)

            if repeats == 1:
                body()
            elif unroll:
                for _ in range(repeats):
                    body()
            else:
                with tc.For_i(0, repeats, 1):
                    body()

            if internal_io:
                done = nc.dram_tensor("done", [P, MT], F32,
                                      kind="ExternalOutput").ap()
                dtile = wpool.tile([P, MT], F32)
                nc.vector.tensor_copy(dtile, bias_sb)
                nc.sync.dma_start(out=done, in_=dtile)

    nc.compile()
    return nc


def _build_module_pg(repeats: int = 1, lead_silu: bool = True,
                     internal_io: bool = False, serial_bf16: bool = True,
                     no_dr_h: bool = False, fuse_o_x: bool = False,
                     only: str | None = None,
                     no_act_dma: bool = False) -> "bacc.Bacc":
    """Per-gate precision build. Gate blocks (new G order, m = bank index):
      m 0..3 = [f0, i0, f1, i1]: ALL 24 k-tiles fp8 DoubleRow
      m 4..5 = [ic0, ic1]:       all bf16 (sin gate is 2-4x more error
                                 sensitive than the sigmoid gates)
      m 6..7 = [o0, o1]:         x k-tiles 0..3 fp8 DR, rest bf16
    Every MM stream alternates PSUM banks (m-inner) — consecutive MMs into
    the same bank measured +130-170ns each (drain serialization / PE
    micro-idle), so k-outer m-inner ordering everywhere.
    Predicted rel_fro: concat 1.60e-2, ht 1.66e-2, ct 1.59e-2 (gate 2e-2)."""
    nc = bacc.Bacc("TRN2", target_bir_lowering=False, debug=False,
                   num_devices=NCORES)
    kin = "Internal" if internal_io else "ExternalInput"
    kout = "Internal" if internal_io else "ExternalOutput"
    KT = KX + KH              # 24 k-tiles total
    OB = 4                    # o-gate bf16 x k-tiles (4..7)
    ODR = 2                   # o-gate fp8 x k-tile pairs

    # Activations stream in bf16 only; the fp8 copies are derived on-device
    # (ACT/DVE casts, pipelined one chunk ahead) — DMA was the binding
    # resource at 6MB/chunk, this cuts it to 3.75MB/chunk. c and the
    # outputs ride bf16 too (adds ~5e-4 rel_fro, negligible).
    xbT = nc.dram_tensor("xbT", [IN, B], BF16, kind=kin).ap()
    hT = nc.dram_tensor("hT", [H, B], BF16, kind=kin).ap()
    cT = nc.dram_tensor("cT", [SH, B], BF16, kind=kin).ap()
    w8fi = nc.dram_tensor("w8fi", [KT * P, 4 * P], F8E4, kind=kin).ap()
    w8o = nc.dram_tensor("w8o", [2 * ODR * P, 2 * P], F8E4, kind=kin).ap()
    wbic = nc.dram_tensor("wbic", [KT * P, 2 * P], BF16, kind=kin).ap()
    wbo = nc.dram_tensor("wbo", [(OB + KH) * P, 2 * P], BF16, kind=kin).ap()
    biasd = nc.dram_tensor("biasd", [P, MT], F32, kind="ExternalInput").ap()
    htT = nc.dram_tensor("htT", [SH, B], BF16, kind=kout).ap()
    ctT = nc.dram_tensor("ctT", [SH, B], BF16, kind=kout).ap()
    if no_dr_h:   # timing-only experiment: f/i h-path in bf16
        assert internal_io
        wbfi = nc.dram_tensor("wbfi", [KH * P, 4 * P], BF16,
                              kind="Internal").ap()
        wbfi3 = wbfi.rearrange("(ko p) g -> p ko g", p=P)

    xbT3 = xbT.rearrange("(ko p) b -> p ko b", p=P)
    hT3 = hT.rearrange("(ko p) b -> p ko b", p=P)
    cT3 = cT.rearrange("(po p) b -> p po b", p=P)
    w8fi3 = w8fi.rearrange("(ko p) g -> p ko g", p=P)   # [128, 24, 512]
    w8o3 = w8o.rearrange("(ko p) g -> p ko g", p=P)     # [128, 4, 256]
    wbic3 = wbic.rearrange("(ko p) g -> p ko g", p=P)   # [128, 24, 256]
    wbo3 = wbo.rearrange("(ko p) g -> p ko g", p=P)     # [128, 20, 256]
    htT3 = htT.rearrange("(po p) b -> p po b", p=P)
    ctT3 = ctT.rearrange("(po p) b -> p po b", p=P)

    PI, TWO_PI = float(np.pi), float(2 * np.pi)
    MUL, ADD = mybir.AluOpType.mult, mybir.AluOpType.add

    with tile.TileContext(nc) as tc:
        with (
            tc.tile_pool(name="wpool", bufs=1) as wpool,
            tc.tile_pool(name="apool", bufs=2) as apool,
            tc.tile_pool(name="gpool", bufs=2) as gpool,
            tc.tile_pool(name="opool", bufs=2) as opool,
            tc.tile_pool(name="pspool", bufs=1, space="PSUM") as pspool,
        ):
            w8fi_t = wpool.tile([P, KT, 4 * P], F8E4, tag="w8fi")
            w8o_t = wpool.tile([P, 2 * ODR, 2 * P], F8E4, tag="w8o")
            wbic_t = wpool.tile([P, KT, 2 * P], BF16, tag="wbic")
            wbo_t = wpool.tile([P, OB + KH, 2 * P], BF16, tag="wbo")
            bias_sb = wpool.tile([P, MT], F32)
            wbfi_t = None
            if no_dr_h:
                wbfi_t = wpool.tile([P, KH, 4 * P], BF16, tag="wbfi")
                for k in range(KH):
                    nc.sync.dma_start(out=wbfi_t[:, k, :], in_=wbfi3[:, k, :])
            for k in range(KT):
                nc.sync.dma_start(out=w8fi_t[:, k, :], in_=w8fi3[:, k, :])
                nc.sync.dma_start(out=wbic_t[:, k, :], in_=wbic3[:, k, :])
            for k in range(2 * ODR):
                nc.sync.dma_start(out=w8o_t[:, k, :], in_=w8o3[:, k, :])
            for k in range(OB + KH):
                nc.sync.dma_start(out=wbo_t[:, k, :], in_=wbo3[:, k, :])
            nc.sync.dma_start(out=bias_sb, in_=biasd)

            if lead_silu:
                # One ACT table set (silu_and_others) for Tanh+Sin; see
                # _build_module.
                dummy = wpool.tile([P, 1], F32)
                nc.vector.memset(dummy, 0.0)
                nc.scalar.activation(dummy, dummy, ACT.Silu)

            # no_act_dma (timing ablation): static activation tiles, no
            # per-chunk streaming — isolates pure PE pitch from DMA limits.
            statics = {}
            if no_act_dma:
                assert internal_io
                statics["xc8"] = wpool.tile([P, KX, NB], F8E4, tag="sxc8",
                                            name="sxc8")
                statics["xcb"] = wpool.tile([P, KX, NB], BF16, tag="sxcb",
                                            name="sxcb")
                statics["hc8"] = wpool.tile([P, KH, NB], F8E4, tag="shc8",
                                            name="shc8")
                statics["hc"] = wpool.tile([P, KH, NB], BF16, tag="shc",
                                           name="shc")
                statics["cc"] = wpool.tile([P, 2, NB], BF16, tag="scc",
                                           name="scc")
                for st in statics.values():
                    nc.vector.memset(st, 0.0)

            def alloc_chunk(nb):
                """DMA the bf16 streams for chunk nb and derive the fp8
                copies on-device (xc8 on ACT, hc8 on DVE). Called one chunk
                ahead so casts overlap the previous chunk's matmuls."""
                bsl = bass.ds(nb * NB, NB)
                t = {}
                t["xcb"] = apool.tile([P, KX, NB], BF16, tag="xcb",
                                      name="xcb")
                t["hc"] = apool.tile([P, KH, NB], BF16, tag="hc", name="hc")
                t["cc"] = apool.tile([P, 2, NB], BF16, tag="cc", name="cc")
                nc.sync.dma_start(out=t["xcb"], in_=xbT3[:, :, bsl])
                nc.sync.dma_start(out=t["hc"], in_=hT3[:, :, bsl])
                nc.sync.dma_start(out=t["cc"], in_=cT3[:, :, bsl])
                t["xc8"] = apool.tile([P, KX, NB], F8E4, tag="xc8",
                                      name="xc8")
                nc.scalar.copy(t["xc8"], t["xcb"])
                if not no_dr_h:
                    t["hc8"] = apool.tile([P, KH, NB], F8E4, tag="hc8",
                                          name="hc8")
                    # split the cast across ACT and DVE to balance engines
                    nc.scalar.copy(t["hc8"][:, :KH // 2, :],
                                   t["hc"][:, :KH // 2, :])
                    nc.vector.tensor_copy(t["hc8"][:, KH // 2:, :],
                                          t["hc"][:, KH // 2:, :])
                return t

            # Prologue: chunk 0's streams+casts are issued before the repeat
            # loop; inside the loop each chunk prefetches its successor, with
            # nb=7 wrapping to chunk 0 for the NEXT loop iteration (8 in-body
            # allocs keep the bufs=2 rotation parity aligned with the
            # prologue buffer).
            state = {"cur": alloc_chunk(0) if not no_act_dma else None}

            def body():
                cur = state["cur"]
                for nb in range(NBCH):
                    bsl = bass.ds(nb * NB, NB)
                    if no_act_dma:
                        xc8, xcb = statics["xc8"], statics["xcb"]
                        hc8, hc = statics["hc8"], statics["hc"]
                        cc = statics["cc"]
                    else:
                        xc8, xcb = cur["xc8"], cur["xcb"]
                        hc8, hc = cur.get("hc8"), cur["hc"]
                        cc = cur["cc"]
                        cur = alloc_chunk((nb + 1) % NBCH)

                    ps = [pspool.tile([P, NB], F32, tag=f"ps{m}", name=f"ps{m}")
                          for m in range(MT)]
                    # `only` is a timing-ablation knob: "fp8" emits just the
                    # DR blocks, "bf16" just the ic/o bf16 blocks; all
                    # elementwise and output DMA is skipped in either mode.
                    do_fp8 = only in (None, "fp8")
                    do_bf16 = only in (None, "bf16")
                    do_ew = only is None
                    if only is not None:
                        assert internal_io

                    # fp8-x block: f/i gates (banks 0-3) + o pairs (6,7).
                    # fuse_o_x folds the o pairs into the f/i kp-rotation
                    # (6-bank rotation for kp 0..1) instead of a trailing
                    # 2-bank segment.
                    def o_dr(kp):
                        for j in range(2):
                            nc.tensor.matmul(
                                ps[6 + j],
                                lhsT=w8o_t[:, 2 * kp:2 * kp + 2,
                                           bass.ds(j * P, P)],
                                rhs=xc8[:, 2 * kp:2 * kp + 2, :],
                                start=(kp == 0),
                                stop=(only == "fp8" and kp == ODR - 1),
                                perf_mode=DR)
                    if do_fp8:
                        for kp in range(KX // 2):
                            for mf in range(4):
                                nc.tensor.matmul(
                                    ps[mf],
                                    lhsT=w8fi_t[:, 2 * kp:2 * kp + 2,
                                                bass.ds(mf * P, P)],
                                    rhs=xc8[:, 2 * kp:2 * kp + 2, :],
                                    start=(kp == 0), stop=False, perf_mode=DR)
                            if fuse_o_x and kp < ODR:
                                o_dr(kp)
                        if not fuse_o_x:
                            for kp in range(ODR):
                                o_dr(kp)
                    # fp8-h block: f/i gates finish here
                    if not do_fp8:
                        pass
                    elif no_dr_h:
                        for k in range(KH):
                            for mf in range(4):
                                nc.tensor.matmul(
                                    ps[mf],
                                    lhsT=wbfi_t[:, k, bass.ds(mf * P, P)],
                                    rhs=hc[:, k, :],
                                    start=False, stop=(k == KH - 1))
                    else:
                        for kp in range(KH // 2):
                            for mf in range(4):
                                nc.tensor.matmul(
                                    ps[mf],
                                    lhsT=w8fi_t[:, KX + 2 * kp:KX + 2 * kp + 2,
                                                bass.ds(mf * P, P)],
                                    rhs=hc8[:, 2 * kp:2 * kp + 2, :],
                                    start=False, stop=(kp == KH // 2 - 1),
                                    perf_mode=DR)

                    # f/i activations (ready first): m = 2*half + {0,1}
                    fi = {}
                    for half in range(2 if do_ew else 0):
                        ft = gpool.tile([P, NB], F32, tag=f"ft{half}")
                        it = gpool.tile([P, NB], F32, tag=f"it{half}")
                        nc.scalar.activation(
                            ft, ps[2 * half], ACT.Tanh,
                            bias=bias_sb[:, 2 * half:2 * half + 1],
                            scale=0.5 / WS)
                        nc.scalar.activation(
                            it, ps[2 * half + 1], ACT.Tanh,
                            bias=bias_sb[:, 2 * half + 1:2 * half + 2],
                            scale=0.5 / WS)
                        # the 0.5x+0.5 affine folds into the consumer muls
                        # via affine_mul_reduce (saves 3 DVE ops per half)
                        fi[half] = (ft, it)

                    # bf16-o block (banks 6,7) runs BEFORE ic so the o
                    # banks' readers drain mid-chunk — the next chunk
                    # re-claims banks 6,7 only ~4us in, while ic banks
                    # (claimed ~23us in) can afford to stop last.
                    def o_mms(j):
                        nc.tensor.matmul(
                            ps[6 + j], lhsT=wbo_t[:, o_k, bass.ds(j * P, P)],
                            rhs=(xcb[:, 2 * ODR + o_k, :] if o_k < OB
                                 else hc[:, o_k - OB, :]),
                            start=(only == "bf16" and o_k == 0),
                            stop=(o_k == OB + KH - 1))
                    if not do_bf16:
                        pass
                    elif serial_bf16:
                        for j in range(2):
                            for o_k in range(OB + KH):
                                o_mms(j)
                    else:
                        for o_k in range(OB + KH):
                            for j in range(2):
                                o_mms(j)

                    # o gate ACT (held for the ht tail)
                    ots = {}
                    for half in range(2 if do_ew else 0):
                        ot = gpool.tile([P, NB], F32, tag=f"ot{half}")
                        nc.scalar.activation(
                            ot, ps[6 + half], ACT.Tanh,
                            bias=bias_sb[:, 6 + half:7 + half],
                            scale=0.5 / WS)
                        ots[half] = ot

                    # bf16-ic block (banks 4,5)
                    def ic_mms(j):
                        nc.tensor.matmul(
                            ps[4 + j], lhsT=wbic_t[:, ic_k, bass.ds(j * P, P)],
                            rhs=(xcb[:, ic_k, :] if ic_k < KX
                                 else hc[:, ic_k - KX, :]),
                            start=(ic_k == 0), stop=(ic_k == KT - 1))
                    if not do_bf16:
                        pass
                    elif serial_bf16:
                        for j in range(2):
                            for ic_k in range(KT):
                                ic_mms(j)
                    else:
                        for ic_k in range(KT):
                            for j in range(2):
                                ic_mms(j)

                    # sin gate + ct chain per half
                    scts = {}
                    for half in range(2 if do_ew else 0):
                        ft, it = fi[half]
                        gw = gpool.tile([P, NB], F32, tag=f"gw{half}")
                        nc.vector.tensor_scalar(
                            gw, ps[4 + half], 1.0 / WS,
                            bias_sb[:, 4 + half:5 + half], MUL, ADD)
                        nc.vector.add_range_wrap(gw, gw, 0.0, PI, TWO_PI)
                        gt = gpool.tile([P, NB], F32, tag=f"gt{half}")
                        nc.scalar.activation(gt, gw, ACT.Sin)
                        ctn = opool.tile([P, NB], F32, tag=f"ctn{half}")
                        tmp = opool.tile([P, NB], F32, tag="tmp")
                        jnk = opool.tile([P, 1], F32, tag=f"jnk{half}",
                                         name="jnk")
                        nc.vector.affine_mul_reduce(
                            ctn, jnk, ft, cc[:, half, :], 0.5, 0.5)
                        nc.vector.affine_mul_reduce(
                            tmp, jnk, it, gt, 0.5, 0.5)
                        nc.vector.tensor_add(ctn, ctn, tmp)
                        cw = opool.tile([P, NB], F32, tag="cw")
                        nc.vector.add_range_wrap(cw, ctn, 0.0, PI, TWO_PI)
                        sct = opool.tile([P, NB], F32, tag=f"sct{half}")
                        nc.scalar.activation(sct, cw, ACT.Sin)
                        ctb = opool.tile([P, NB], BF16, tag=f"ctb{half}")
                        nc.vector.tensor_copy(ctb, ctn)
                        nc.sync.dma_start(out=ctT3[:, half, bsl], in_=ctb)
                        scts[half] = sct

                    # ht tail (needs sct from the ct chain; ot was computed
                    # mid-chunk and held)
                    for half in range(2 if do_ew else 0):
                        htn = opool.tile([P, NB], BF16, tag=f"htn{half}")
                        jnkh = opool.tile([P, 1], F32, tag=f"jnkh{half}",
                                          name="jnkh")
                        nc.vector.affine_mul_reduce(
                            htn, jnkh, ots[half], scts[half], 0.5, 0.5)
                        nc.sync.dma_start(out=htT3[:, half, bsl], in_=htn)

            if repeats == 1:
                body()
            else:
                with tc.For_i(0, repeats, 1):
                    body()

            if internal_io:
                done = nc.dram_tensor("done", [P, MT], F32,
                                      kind="ExternalOutput").ap()
                dtile = wpool.tile([P, MT], F32)
                nc.vector.tensor_copy(dtile, bias_sb)
                nc.sync.dma_start(out=done, in_=dtile)

    nc.compile()
    return nc


def make_in_maps_pg(x, h, c, w_ih, w_hh, b_ih, b_hh):
    """Host-side shard/transpose/quantize for the per-gate build."""
    x = np.asarray(x, np.float32)
    h = np.asarray(h, np.float32)
    c = np.asarray(c, np.float32)
    w_ih = np.asarray(w_ih, np.float32)
    w_hh = np.asarray(w_hh, np.float32)
    bias = np.asarray(b_ih, np.float32) + np.asarray(b_hh, np.float32)

    f8np = mybir.dt.np(F8E4)
    bf = ml_dtypes.bfloat16
    xbT = np.ascontiguousarray(x.T).astype(bf)
    hT8b = np.ascontiguousarray(h.T).astype(bf)
    cTt = np.ascontiguousarray(c.T).astype(bf)
    Wfull = np.concatenate([w_ih, w_hh], axis=1)  # [4H, IN+H]

    # m/bank order: [f0, i0, f1, i1, ic0, ic1, o0, o1]; mscale 0.5 for
    # tanh-based sigmoid gates, 1.0 for the sin gate.
    order = [(0, 0), (1, 0), (0, 1), (1, 1), (2, 0), (2, 1), (3, 0), (3, 1)]
    mscale = np.array([.5, .5, .5, .5, 1., 1., .5, .5], np.float32)

    in_maps = []
    for core in range(NCORES):
        cols = np.concatenate(
            [gate * H + core * SH + half * P + np.arange(P)
             for gate, half in order])
        W_cT = np.ascontiguousarray(Wfull[cols].T) * np.float32(WS)  # [3072,G]
        w8fi = W_cT[:, 0:4 * P].astype(f8np)
        w8o = W_cT[0:4 * P, 6 * P:8 * P].astype(f8np)
        wbic = np.ascontiguousarray(W_cT[:, 4 * P:6 * P]).astype(bf)
        wbo = np.ascontiguousarray(
            W_cT[4 * P:, 6 * P:8 * P]).astype(bf)  # x k4..7 + h k0..15
        b_c = bias[cols]
        bias_mat = np.ascontiguousarray(
            (b_c.reshape(MT, P) * mscale[:, None]).T)
        in_maps.append({
            "xbT": xbT,
            "hT": hT8b,
            "cT": np.ascontiguousarray(cTt[core * SH:(core + 1) * SH]),
            "w8fi": w8fi,
            "w8o": w8o,
            "wbic": wbic,
            "wbo": wbo,
            "biasd": bias_mat,
        })
    return in_maps


def _get_module(repeats: int = 1) -> "bacc.Bacc":
    if repeats not in _MODULES:
        _MODULES[repeats] = _build_module_pg(repeats)
    return _MODULES[repeats]


def make_in_maps(x, h, c, w_ih, w_hh, b_ih, b_hh):
    """Host-side shard + transpose. Returns the per-core input maps."""
    x = np.asarray(x, np.float32)
    h = np.asarray(h, np.float32)
    c = np.asarray(c, np.float32)
    w_ih = np.asarray(w_ih, np.float32)
    w_hh = np.asarray(w_hh, np.float32)
    bias = np.asarray(b_ih, np.float32) + np.asarray(b_hh, np.float32)

    f8np = mybir.dt.np(F8E4)
    xT = np.ascontiguousarray(x.T)                               # [IN, B]
    x8T = xT[:NFP8 * P].astype(f8np)
    xbT = np.ascontiguousarray(xT[NFP8 * P:]).astype(ml_dtypes.bfloat16)
    hTt = np.ascontiguousarray(h.T).astype(ml_dtypes.bfloat16)   # [H, B]
    cTt = np.ascontiguousarray(c.T)                              # [H, B] f32

    # m-tile scale: 0.5 for tanh-based sigmoid gates (f,i,o), 1.0 for sin (ic)
    # m ordering is [f0,i0,ic0,o0, f1,i1,ic1,o1] (m = gate + 4*half)
    mscale = np.array([0.5, 0.5, 1.0, 0.5, 0.5, 0.5, 1.0, 0.5], np.float32)

    in_maps = []
    for core in range(NCORES):
        rows = np.concatenate(
            [gate * H + core * SH + half * P + np.arange(P)
             for half in range(2) for gate in range(4)])
        wihT_c = np.ascontiguousarray(w_ih[rows].T) * np.float32(WS)  # [IN,G]
        wih8_c = wihT_c[:NFP8 * P].astype(f8np)
        wihb_c = np.ascontiguousarray(
            wihT_c[NFP8 * P:]).astype(ml_dtypes.bfloat16)
        whh_c = (np.ascontiguousarray(w_hh[rows].T)
                 * np.float32(WS)).astype(ml_dtypes.bfloat16)
        b_c = bias[rows]                             # [G]
        bias_mat = np.ascontiguousarray(
            (b_c.reshape(MT, P) * mscale[:, None]).T)  # [P, MT]
        in_maps.append({
            "x8T": x8T,
            "xbT": xbT,
            "hT": hTt,
            "cT": np.ascontiguousarray(cTt[core * SH:(core + 1) * SH]),
            "wih8": wih8_c,
            "wihb": wihb_c,
            "whh": whh_c,
            "biasd": bias_mat,
        })
    return in_maps


def assemble_outputs(results):
    """results: per-core dicts with htT/ctT [SH, B] bf16 -> full f32."""
    htT = np.concatenate([results[c]["htT"] for c in range(NCORES)], axis=0)
    ctT = np.concatenate([results[c]["ctT"] for c in range(NCORES)], axis=0)
    ht = np.ascontiguousarray(htT.T).astype(np.float32)
    ct = np.ascontiguousarray(ctT.T).astype(np.float32)
    return ht, ct


def kernel(x, h, c, w_ih, w_hh, b_ih, b_hh):
    nc = _get_module(repeats=1)
    in_maps = make_in_maps_pg(x, h, c, w_ih, w_hh, b_ih, b_hh)
    res = run_bass_kernel_spmd(nc, in_maps, core_ids=list(range(NCORES)))
    return assemble_outputs(res.results)
